# revision 13
# baseline (speedup 1.0000x reference)
"""AttentionalPooler Trainium2 kernel (v2: fp8 DoubleRow kv projection).

Full inputs -> full output; batch (8) is data-parallel across the 8
NeuronCores. Per core: LayerNorm(x_b), kv = LN(x_b) @ Wkv, 12-head
cross-attention from 256 pre-computed queries, output projection.

Numeric scheme (validated host-side, rel err ~0.010 vs 2e-2 gate):
  - LN(x) is quantized to a (hi, lo) fp8e4 pair per element: hi = q8(xn),
    lo = q8(xn - hi). The pair is byte-packed into one uint16 per element
    so the bf16-style xbar DMA transpose (2-byte) moves both at once, and
    the byte-pair lands exactly in the [K, 2, N] slot layout that fp8
    DoubleRow matmuls (0.5 cycles/col) consume.
  - k-projection: 2-term (Whi + Wlo) x hi  -> 0.5x bf16 PE cost, weight
    quantization error cancelled by the host-side hi/lo weight split.
  - v-projection: 3-term Whi x hi + Whi x lo + Wlo x hi -> 0.75x bf16 PE
    cost, both weight- and activation-quantization errors cancelled.
  - sim: bf16, block-diagonal q so one matmul covers a 2-head pair
    (N=512 keeps the PE sequencer off the critical path).
  - rstd = 1/sqrt(var+eps) via bit-magic + 2 Newton steps entirely on
    DVE, batched per quarter: no ACT Sqrt, so the Exp activation table
    is loaded exactly once (table swaps cost 1.3us each).
  - exp / attn / attn@v / out-projection: bf16 (fp8 there fails the
    accuracy gate). Softmax denominators from a ones-column in v (M=65).

Engine placement: GPSIMD (Pool) runs both fp8 cast passes (it is
otherwise idle), ACT drains the k psums + exp, DVE does stats, bf16
normalize (4x mode), v drains, attn@v accumulation and the epilogue.

Host-side preprocessing (exact fp32 algebra, batch-independent): q path
(LN(query) @ Wq * dh^-0.5 / S, block-diagonal by head pair), ln_k_w/b
folded into Wkv (softmax cancels the k-bias shift; the v-bias term
commutes to a constant r = c_v @ Wout added at the end), Wkv scaled by
S=32 and split hi/lo into the DoubleRow slot layouts, Wout/S repacked
by head pair for K=128 output-projection matmuls.
"""

import sys

sys.path.insert(0, "/opt/trn_rl_repo")

import numpy as np
import ml_dtypes

import concourse.bass as bass
import concourse.mybir as mybir
import concourse.tile as tile
from concourse import bacc
from concourse.bass_utils import run_bass_kernel_spmd

F32 = mybir.dt.float32
BF16 = mybir.dt.bfloat16
FP8 = mybir.dt.float8e4
U16 = mybir.dt.uint16
I32 = mybir.dt.int32
AX = mybir.AluOpType
DR = mybir.MatmulPerfMode.DoubleRow
ACTF = mybir.ActivationFunctionType

B = 8
N_TOK = 4096
D_CTX = 1024
D_MODEL = 768
N_HEAD = 12
DH = 64
NQ = 256
INNER = 768
EPS = 1e-5
N_CORES = 8
S = 32.0

TOK_TILES = N_TOK // 128  # 32
D_CHUNKS = D_CTX // 128  # 8 (u16 transpose chunks == k-tiles)
E_TILES = INNER // 128  # 6 head pairs
MAGIC = 0x5F3759DF

QSIZES = [2, 2, 4, 4, 4, 4, 4, 4, 4]
assert sum(QSIZES) == TOK_TILES


def emit_kernel(ctx, tc, out_d, x_d, wkhi_d, wklo_d, wvhi_d, wvlo_d, qtbd_d,
                woutp_d, rrep_d, rep=0):
    nc = tc.nc
    xq_dram = nc.dram_tensor(f"xq_scratch{rep}", [N_TOK, D_CTX], U16).ap()

    p_w = ctx.enter_context(tc.tile_pool(name="w", bufs=1))
    p_x = ctx.enter_context(tc.tile_pool(name="x", bufs=8))
    p_xn = ctx.enter_context(tc.tile_pool(name="xn", bufs=3))
    p_xq = ctx.enter_context(tc.tile_pool(name="xq", bufs=3))
    p_xqt = ctx.enter_context(tc.tile_pool(name="xqt", bufs=2))
    p_kt = ctx.enter_context(tc.tile_pool(name="kt", bufs=2))
    p_v = ctx.enter_context(tc.tile_pool(name="v", bufs=2))
    p_attn = ctx.enter_context(tc.tile_pool(name="attn", bufs=E_TILES + 1))
    p_acc = ctx.enter_context(tc.tile_pool(name="acc", bufs=N_HEAD))
    p_stat = ctx.enter_context(tc.tile_pool(name="stat", bufs=10))
    p_ot = ctx.enter_context(tc.tile_pool(name="ot", bufs=E_TILES))
    p_fin = ctx.enter_context(tc.tile_pool(name="fin", bufs=2))
    p_rc = ctx.enter_context(tc.tile_pool(name="rc", bufs=2))
    ps_kv = ctx.enter_context(tc.tile_pool(name="pskv", bufs=3, space="PSUM"))
    ps_sim = ctx.enter_context(tc.tile_pool(name="pssim", bufs=3, space="PSUM"))
    ps_av = ctx.enter_context(tc.tile_pool(name="psav", bufs=2, space="PSUM"))

    # persistent per-token stats: [mean, var] per tile, rstd per tile
    mv_all = p_stat.tile([128, TOK_TILES, 2], F32, tag="mv")
    rstd_all = p_stat.tile([128, TOK_TILES], F32, tag="rstd")

    warm_ps = ps_av.tile([128, 256], F32, tag="ps", name="warm")

    x_tiles = {}

    def prep_tile(i, warm=False):
        xt = p_x.tile([128, D_CTX], BF16, tag="x", name=f"x{i}")
        x_tiles[i] = xt
        nc.sync.dma_start(out=xt[:], in_=x_d[i * 128:(i + 1) * 128, :])
        st = p_stat.tile([128, 2, 6], F32, tag="st", name=f"st{i}")
        nc.vector.bn_stats(out=st[:, 0, :], in_=xt[:, 0:512])
        nc.vector.bn_stats(out=st[:, 1, :], in_=xt[:, 512:1024])
        nc.vector.bn_aggr(out=mv_all[:, i, :], in_=st[:])
        if warm:
            # dummy matmuls chained on this tile keep the PE p-state warm
            # through the DMA/cast-heavy pipeline fill
            for _ in range(4):
                nc.tensor.matmul(
                    out=warm_ps[:],
                    lhsT=xt[:, 0:128],
                    rhs=xt[:, 0:256],
                    start=True,
                    stop=True,
                )

    def newton_rstd(j0, nj):
        # rstd[:, j0:j0+nj] = 1/sqrt(var + eps), batched on DVE
        a = p_stat.tile([128, nj], F32, tag="nta", name=f"nta{j0}")
        nc.vector.tensor_scalar(
            out=a[:], in0=mv_all[:, j0:j0 + nj, 1], scalar1=EPS, scalar2=None,
            op0=AX.add,
        )
        y = rstd_all[:, j0:j0 + nj]
        # seed: y = bitcast(MAGIC - (bitcast_i32(a) >> 1))
        nc.vector.tensor_scalar(
            out=y.bitcast(I32),
            in0=a[:].bitcast(I32),
            scalar1=1,
            scalar2=None,
            op0=AX.logical_shift_right,
        )
        nc.vector.tensor_scalar(
            out=y.bitcast(I32), in0=y.bitcast(I32), scalar1=-1, op0=AX.mult,
            scalar2=MAGIC, op1=AX.add,
        )
        u = p_stat.tile([128, nj], F32, tag="ntu", name=f"ntu{j0}")
        for _ in range(2):
            # u = 1.5 - 0.5*a*y^2 ; y *= u
            nc.vector.tensor_tensor(out=u[:], in0=y, in1=y, op=AX.mult)
            nc.vector.tensor_tensor(out=u[:], in0=u[:], in1=a[:], op=AX.mult)
            nc.vector.tensor_scalar(
                out=u[:], in0=u[:], scalar1=-0.5, op0=AX.mult, scalar2=1.5,
                op1=AX.add,
            )
            nc.vector.tensor_tensor(out=y, in0=y, in1=u[:], op=AX.mult)

    def cast_tile(i):
        # xn (bf16, DVE 4x) then hi/lo fp8 bytes into the u16 stage (Pool)
        xt = x_tiles.pop(i)
        mu = mv_all[:, i, 0:1]
        rs = rstd_all[:, i:i + 1]
        xn = p_xn.tile([128, D_CTX], BF16, tag="xn", name=f"xn{i}")
        nc.vector.tensor_scalar(
            out=xn[:], in0=xt[:], scalar1=mu, scalar2=rs, op0=AX.subtract,
            op1=AX.mult,
        )
        xq = p_xq.tile([128, D_CTX], U16, tag="xq", name=f"xq{i}")
        xq8 = xq[:].bitcast(FP8).rearrange("p (n two) -> p two n", two=2)
        nc.gpsimd.tensor_scalar(
            out=xq8[:, 0, :], in0=xt[:], scalar1=mu, scalar2=rs,
            op0=AX.subtract, op1=AX.mult,
        )
        nc.gpsimd.tensor_tensor(
            out=xq8[:, 1, :], in0=xn[:], in1=xq8[:, 0, :], op=AX.subtract
        )
        nc.sync.dma_start(out=xq_dram[i * 128:(i + 1) * 128, :], in_=xq[:])

    # --- prologue: stats+casts for the first two quarters, weight loads ---
    qstarts = [sum(QSIZES[:g]) for g in range(len(QSIZES))]
    for i in range(qstarts[2]):
        prep_tile(i, warm=(i < 4))

    wkhi = p_w.tile([128, 4, 2, INNER], FP8, tag="wkhi")
    nc.gpsimd.dma_start(out=wkhi[:], in_=wkhi_d[:, :, :, :])
    qtbd = p_w.tile([128, E_TILES, 2 * NQ], BF16, tag="qtbd")
    nc.gpsimd.dma_start(out=qtbd[:], in_=qtbd_d[:, :, :])
    wklo = p_w.tile([128, 4, 2, INNER], FP8, tag="wklo")
    nc.gpsimd.dma_start(out=wklo[:], in_=wklo_d[:, :, :, :])
    wvhi = p_w.tile([128, 4, 2, INNER], FP8, tag="wvhi")
    nc.gpsimd.dma_start(out=wvhi[:], in_=wvhi_d[:, :, :, :])
    wvlo = p_w.tile([128, 4, 2, INNER], FP8, tag="wvlo")
    nc.gpsimd.dma_start(out=wvlo[:], in_=wvlo_d[:, :, :, :])
    rrep = p_w.tile([128, D_MODEL], F32, tag="rrep")
    nc.gpsimd.dma_start(out=rrep[:], in_=rrep_d[:, :])
    ones_t = p_w.tile([128, DH], F32, tag="ones")
    nc.vector.memset(ones_t[:], 1.0)

    for g in range(2):
        newton_rstd(qstarts[g], QSIZES[g])
        for i in range(qstarts[g], qstarts[g] + QSIZES[g]):
            cast_tile(i)

    # extra PE warm-up during the fill window
    warm = p_w.tile([128, 256], BF16, tag="warm")
    nc.vector.memset(warm[:], 1.0)
    for _ in range(30):
        nc.tensor.matmul(
            out=warm_ps[:], lhsT=warm[:, 0:128], rhs=warm[:], start=True,
            stop=True,
        )

    av_acc = []
    for h in range(N_HEAD):
        av_acc.append(p_acc.tile([DH + 1, NQ], F32, tag="acc", name=f"acc{h}"))
    ot_pairs = []
    for e in range(E_TILES):
        ot_pairs.append(p_ot.tile([128, NQ], BF16, tag="ot", name=f"ot{e}"))

    # --- main loop over quarters ---------------------------------------
    prepped = qstarts[2]
    casted = qstarts[2]
    n_q = len(QSIZES)
    for q, (j0, nj) in enumerate(zip(qstarts, QSIZES)):
        last_q = q == n_q - 1
        ntok = nj * 128

        # stream prep+cast for quarter q+2 while computing q (quarters 0/1
        # were handled in the prologue)
        if q + 2 < n_q:
            g = q + 2
            for i in range(qstarts[g], qstarts[g] + QSIZES[g]):
                prep_tile(i)
                prepped += 1
            newton_rstd(qstarts[g], QSIZES[g])
            for i in range(qstarts[g], qstarts[g] + QSIZES[g]):
                cast_tile(i)
                casted += 1

        # transpose this quarter's bounced u16 data: chunk c -> partitions
        xqt = p_xqt.tile([128, D_CHUNKS, ntok], U16, tag="xqt", name=f"xqt{q}")
        for c in range(D_CHUNKS):
            nc.sync.dma_start(
                out=xqt[:, c, :],
                in_=xq_dram[j0 * 128:(j0 + nj) * 128, c * 128:(c + 1) * 128],
                transpose=True,
            )
        # fp8 views of the packed tile
        xq8 = xqt[:].bitcast(FP8).rearrange(
            "p c (n two) -> p c n two", two=2
        )  # [128, chunk, tok, byte]

        # --- k projection: psum [128, ntok] per e, 8 DoubleRow instrs ----
        kt = p_kt.tile([128, E_TILES, ntok], BF16, tag="kt", name=f"kt{q}")
        for e in range(E_TILES):
            for n2 in range(0, ntok, 512):
                nw = min(512, ntok - n2)
                ps = ps_kv.tile([128, 512], F32, tag="ps", name=f"pk{q}_{e}_{n2}")
                rhs = xq8[:, :, n2:n2 + nw, 0]  # [128, chunk, nw] hi bytes
                for dp in range(4):
                    nc.tensor.matmul(
                        out=ps[:, 0:nw],
                        lhsT=wkhi[:, dp, :, e * 128:(e + 1) * 128],
                        rhs=rhs[:, 2 * dp:2 * dp + 2, :],
                        start=(dp == 0),
                        stop=False,
                        perf_mode=DR,
                    )
                for dp in range(4):
                    nc.tensor.matmul(
                        out=ps[:, 0:nw],
                        lhsT=wklo[:, dp, :, e * 128:(e + 1) * 128],
                        rhs=rhs[:, 2 * dp:2 * dp + 2, :],
                        start=False,
                        stop=(dp == 3),
                        perf_mode=DR,
                    )
                nc.scalar.activation(
                    out=kt[:, e, n2:n2 + nw], in_=ps[:, 0:nw], func=ACTF.Copy
                )

        # --- sim + exp + v projection, interleaved per token tile --------
        # (exp on ACT is ~3x slower than the sim matmul; interleaving the
        # fp8 v-projection keeps the PE busy while ACT drains)
        attn_tiles = []
        for e in range(E_TILES):
            attn_tiles.append(
                p_attn.tile([128, nj, 2, NQ], BF16, tag="attn", name=f"at{q}_{e}")
            )
        vbig = p_v.tile([128, nj, N_HEAD, DH + 1], BF16, tag="v", name=f"v{q}")
        nc.vector.memset(vbig[:, :, :, DH:DH + 1], 1.0)
        for jj in range(nj):
            for e in range(E_TILES):
                ps = ps_sim.tile([128, 2, NQ], F32, tag="ps", name=f"psim{q}_{e}_{jj}")
                nc.tensor.matmul(
                    out=ps[:].rearrange("p a b -> p (a b)"),
                    lhsT=kt[:, e, jj * 128:(jj + 1) * 128],
                    rhs=qtbd[:, e, :],
                    start=True,
                    stop=True,
                )
                nc.scalar.activation(
                    out=attn_tiles[e][:, jj, :, :], in_=ps[:], func=ACTF.Exp
                )
            tsl = slice(jj * 128, (jj + 1) * 128)
            # 3 chunk-paired DoubleRow instrs per d-pair: hi@Whi, lo@Whi,
            # hi@Wlo (slot stride must be large: byte-slot lhsT fails the
            # s3_lw_dual_fp8 ISA check). Output split 512+256 to stay
            # within one PSUM bank per matmul.
            for c0, cw in ((0, 512), (512, 256)):
                ps = ps_kv.tile([128, 512], F32, tag="ps", name=f"pv{q}_{jj}_{c0}")
                for dp in range(4):
                    hi = xq8[:, 2 * dp:2 * dp + 2, tsl, 0]  # [128, 2, 128]
                    lo = xq8[:, 2 * dp:2 * dp + 2, tsl, 1]
                    nc.tensor.matmul(
                        out=ps[:, 0:cw], lhsT=hi,
                        rhs=wvhi[:, dp, :, c0:c0 + cw],
                        start=(dp == 0), stop=False, perf_mode=DR,
                    )
                    nc.tensor.matmul(
                        out=ps[:, 0:cw], lhsT=lo,
                        rhs=wvhi[:, dp, :, c0:c0 + cw],
                        start=False, stop=False, perf_mode=DR,
                    )
                    nc.tensor.matmul(
                        out=ps[:, 0:cw], lhsT=hi,
                        rhs=wvlo[:, dp, :, c0:c0 + cw],
                        start=False, stop=(dp == 3), perf_mode=DR,
                    )
                nc.vector.tensor_copy(
                    out=vbig[:, jj, c0 // DH:(c0 + cw) // DH, 0:DH],
                    in_=ps[:, 0:cw].rearrange("p (h dh) -> p h dh", dh=DH),
                )

        # --- attn @ v, accumulate per head ------------------------------
        for e in range(E_TILES):
            for hh in range(2):
                h = 2 * e + hh
                psa = ps_av.tile([DH + 1, NQ], F32, tag="ps", name=f"pav{q}_{h}")
                for jj in range(nj):
                    nc.tensor.matmul(
                        out=psa[:],
                        lhsT=vbig[:, jj, h, :],
                        rhs=attn_tiles[e][:, jj, hh, :],
                        start=(jj == 0),
                        stop=(jj == nj - 1),
                    )
                if q == 0:
                    nc.vector.tensor_copy(out=av_acc[h][:], in_=psa[:])
                else:
                    nc.vector.tensor_tensor(
                        out=av_acc[h][:], in0=av_acc[h][:], in1=psa[:], op=AX.add
                    )
                if last_q:
                    # normalize: reciprocal of the ones-row, partition-
                    # broadcast via a K=1 matmul, multiply into ot pair
                    rc_sb = p_rc.tile([128, NQ], F32, tag="rc", name=f"rc{h}")
                    nc.vector.reciprocal(
                        out=rc_sb[DH:DH + 1, :], in_=av_acc[h][DH:DH + 1, :]
                    )
                    ps_rc = ps_sim.tile([DH, NQ], F32, tag="ps", name=f"psrc{h}")
                    nc.tensor.matmul(
                        out=ps_rc[:],
                        lhsT=ones_t[DH:DH + 1, 0:DH],
                        rhs=rc_sb[DH:DH + 1, :],
                        start=True,
                        stop=True,
                    )
                    nc.vector.tensor_tensor(
                        out=ot_pairs[e][hh * DH:(hh + 1) * DH, :],
                        in0=av_acc[h][0:DH, :],
                        in1=ps_rc[:],
                        op=AX.mult,
                    )

    # wout loads into the freed wvhi slot region late
    woutp = p_w.tile([128, E_TILES, D_MODEL], BF16, tag="woutp")
    nc.gpsimd.dma_start(out=woutp[:], in_=woutp_d[:, :, :])

    # --- output projection: K=128 head pairs ----------------------------
    for q2 in range(NQ // 128):
        fin = p_fin.tile([128, D_MODEL], F32, tag="fin", name=f"fin{q2}")
        for n2 in range(2):
            psf = ps_kv.tile([128, 384], F32, tag="ps", name=f"pf{q2}_{n2}")
            for e in range(E_TILES):
                nc.tensor.matmul(
                    out=psf[:],
                    lhsT=ot_pairs[e][:, q2 * 128:(q2 + 1) * 128],
                    rhs=woutp[:, e, n2 * 384:(n2 + 1) * 384],
                    start=(e == 0),
                    stop=(e == E_TILES - 1),
                )
            nc.vector.tensor_tensor(
                out=fin[:, n2 * 384:(n2 + 1) * 384],
                in0=psf[:],
                in1=rrep[:, n2 * 384:(n2 + 1) * 384],
                op=AX.add,
            )
        nc.sync.dma_start(out=out_d[q2 * 128:(q2 + 1) * 128, :], in_=fin[:])


def build_nc(reps=1):
    nc = bacc.Bacc(
        "TRN2", target_bir_lowering=False, debug=False, num_devices=N_CORES
    )
    x_d = nc.dram_tensor("x", [N_TOK, D_CTX], BF16, kind="ExternalInput").ap()
    wkhi_d = nc.dram_tensor("wkhi", [128, 4, 2, INNER], FP8, kind="ExternalInput").ap()
    wklo_d = nc.dram_tensor("wklo", [128, 4, 2, INNER], FP8, kind="ExternalInput").ap()
    wvhi_d = nc.dram_tensor("wvhi", [128, 4, 2, INNER], FP8, kind="ExternalInput").ap()
    wvlo_d = nc.dram_tensor("wvlo", [128, 4, 2, INNER], FP8, kind="ExternalInput").ap()
    qtbd_d = nc.dram_tensor(
        "qtbd", [128, E_TILES, 2 * NQ], BF16, kind="ExternalInput"
    ).ap()
    woutp_d = nc.dram_tensor(
        "woutp", [128, E_TILES, D_MODEL], BF16, kind="ExternalInput"
    ).ap()
    rrep_d = nc.dram_tensor("rrep", [128, D_MODEL], F32, kind="ExternalInput").ap()
    out_d = nc.dram_tensor("out", [NQ, D_MODEL], F32, kind="ExternalOutput").ap()
    from contextlib import ExitStack

    with tile.TileContext(nc) as tc:
        for rep in range(reps):
            with ExitStack() as ctx:
                emit_kernel(
                    ctx, tc, out_d, x_d, wkhi_d, wklo_d, wvhi_d, wvlo_d,
                    qtbd_d, woutp_d, rrep_d, rep=rep,
                )
    nc.compile()
    return nc


def host_prep(query, ln_q_w, ln_q_b, ln_k_w, ln_k_b, Wq, Wkv, Wout):
    """Batch-independent fp32 preprocessing -> per-core input dict (minus x)."""
    F8NP = ml_dtypes.float8_e4m3
    query = np.asarray(query, np.float32)
    mu = query.mean(-1, keepdims=True)
    var = ((query - mu) ** 2).mean(-1, keepdims=True)
    qn = (query - mu) / np.sqrt(var + EPS) * ln_q_w + ln_q_b
    qmat = (qn @ np.asarray(Wq, np.float32)) * (DH ** -0.5)
    qT = (qmat.T / S).astype(ml_dtypes.bfloat16).astype(np.float32)  # [INNER, NQ]

    # block-diagonal per head pair: [128, 6, 512]
    qtbd = np.zeros((128, E_TILES, 2 * NQ), np.float32)
    for e in range(E_TILES):
        qtbd[0:64, e, 0:NQ] = qT[e * 128:e * 128 + 64, :]
        qtbd[64:128, e, NQ:2 * NQ] = qT[e * 128 + 64:(e + 1) * 128, :]
    qtbd = qtbd.astype(ml_dtypes.bfloat16)

    Wkv = np.asarray(Wkv, np.float32)
    WpS = (np.asarray(ln_k_w, np.float32)[:, None] * Wkv) * S  # [1024, 1536]
    Whi = WpS.astype(F8NP)
    Wlo = (WpS - Whi.astype(np.float32)).astype(F8NP)

    Wk_hi, Wk_lo = Whi[:, :INNER], Wlo[:, :INNER]
    Wv_hi, Wv_lo = Whi[:, INNER:], Wlo[:, INNER:]

    # k: chunk pairs (2dp+i)*128+p
    wkhi = np.zeros((128, 4, 2, INNER), F8NP)
    wklo = np.zeros((128, 4, 2, INNER), F8NP)
    for dp in range(4):
        for i in range(2):
            rows = slice((2 * dp + i) * 128, (2 * dp + i + 1) * 128)
            wkhi[:, dp, i, :] = Wk_hi[rows, :]
            wklo[:, dp, i, :] = Wk_lo[rows, :]
    # v hi: chunk pairs, same layout as the lo parts
    wvhi = np.zeros((128, 4, 2, INNER), F8NP)
    for dp in range(4):
        for i in range(2):
            rows = slice((2 * dp + i) * 128, (2 * dp + i + 1) * 128)
            wvhi[:, dp, i, :] = Wv_hi[rows, :]
    # v lo: chunk pairs
    wvlo = np.zeros((128, 4, 2, INNER), F8NP)
    for dp in range(4):
        for i in range(2):
            rows = slice((2 * dp + i) * 128, (2 * dp + i + 1) * 128)
            wvlo[:, dp, i, :] = Wv_lo[rows, :]

    c = np.asarray(ln_k_b, np.float32) @ Wkv
    Wout = np.asarray(Wout, np.float32)
    r = c[INNER:] @ Wout
    rrep = np.ascontiguousarray(np.broadcast_to(r, (128, D_MODEL))).astype(np.float32)

    WoS = Wout / S  # [INNER, D_MODEL]
    woutp = np.zeros((128, E_TILES, D_MODEL), np.float32)
    for e in range(E_TILES):
        woutp[:, e, :] = WoS[e * 128:(e + 1) * 128, :]
    woutp = woutp.astype(ml_dtypes.bfloat16)

    return {
        "wkhi": wkhi, "wklo": wklo, "wvhi": wvhi, "wvlo": wvlo,
        "qtbd": qtbd, "woutp": woutp, "rrep": rrep,
    }


_NC_CACHE = {}


def get_nc():
    if "nc" not in _NC_CACHE:
        _NC_CACHE["nc"] = build_nc()
    return _NC_CACHE["nc"]


def kernel(x, query, ln_q_w, ln_q_b, ln_k_w, ln_k_b, Wq, Wkv, Wout):
    x = np.asarray(x, np.float32)
    shared = host_prep(query, ln_q_w, ln_q_b, ln_k_w, ln_k_b, Wq, Wkv, Wout)
    in_maps = [
        {"x": np.ascontiguousarray(x[b]).astype(ml_dtypes.bfloat16), **shared}
        for b in range(B)
    ]
    nc = get_nc()
    res = run_bass_kernel_spmd(nc, in_maps, list(range(N_CORES)))
    return np.stack([res.results[b]["out"] for b in range(B)], axis=0)
